# revision 14
# baseline (speedup 1.0000x reference)
"""KAN layer (B-spline + SiLU) Trainium2 kernel.

Math: y[b,k] = scale * sum_i( silu(x[b,i])*W[i,k]
                              + sum_j basis_j(x[b,i]) * C[i,k,j] )

With a uniform grid (12 knots, spacing h, first knot g0), the cubic
B-spline basis is  basis_j(x) = B3(u - j),  u = (x - g0)/h, u in [0,11],
and B3 expands into truncated relu-cubes r_s = relu(u-s)^3:
  B3(u-j) = sum_m (-1)^m C(4,m)/6 * r_{j+m}
x is clamped to the grid ends (u in [0,11]) where every basis fn is
exactly 0, which handles out-of-range inputs.

The truncated-power basis cancels catastrophically under low precision,
and exact-fp32 matmuls cost 4 cyc/row vs 1 for tf32 (float32r). So the
large cubes are mirrored:  r_s = (u-s)^3 + rho_s,  rho_s = relu(s-u)^3
for s=0..5. The polynomial parts fold into three shared monomial chunks
u, u^2, u^3 (large weights -> exact fp32) plus a per-output bias added
at PSUM evacuation; the remaining cube features rho_1..rho_5, r_6..r_10
are bounded by ~125, so all but the two biggest (rho_5, r_6, kept exact)
run as tf32 matmuls. Measured rel err vs fp64 reference: ~7e-3 (budget
2e-2).

Feature production is spread over three engines and split per 512-column
half so half 0's PSUM chain closes early and its evacuation + output DMA
overlap half 1's compute:
  DVE:    x-clamp + 8 fused affine-relu-cube ops (rho_1..4, r_7..10)
  Scalar: silu, u, u^2, and Square/Relu pairs for rho_5/r_6
  GpSimd: the three tensor_tensor products u^3=u^2*u, rho_5, r_6
The matmul chain is issued in feature-availability order with same-dtype
chunks grouped (fp32<->f32r mode switches cost a pipeline restart) and a
cheap tf32 chunk as the tail. Dummy matmuls during the input-DMA wait
pre-warm the PE's HAM clock gate.

Sharding: data-parallel over batch, 1024 rows per core on 8 cores.
"""

import math
import os
import sys

import numpy as np

if "/opt/trn_rl_repo" not in sys.path:
    sys.path.insert(0, "/opt/trn_rl_repo")

import concourse.bass as bass
import concourse.mybir as mybir
from concourse import bacc
from concourse.tile import TileContext

B_TOTAL = 8192
IN_DIM = 128
OUT_DIM = 128
N_CORES = 8
B_CORE = B_TOTAL // N_CORES  # 1024
NB = 8    # num basis
NS = 11   # truncated-cube shifts s=0..10
NMIR = 6  # shifts s=0..5 are mirrored (rho_0 is identically 0)
NEF = 5   # exact chunks: u, u^2, u^3, rho_5, r_6
NRF = 9   # tf32 chunks: silu, rho_1..4, r_7..10
HB = 512  # column half

F32 = mybir.dt.float32
F32R = mybir.dt.float32r
AF = mybir.ActivationFunctionType

# ---------------------------------------------------------------- custom DVE op


def _register_ops():
    from concourse.dve_ops import (
        _CUSTOM_DVE_ROW_BASE,
        _SUB_OPCODE_FOR_NAME,
        CUSTOM_DVE_SPECS,
        OPS,
        DveOp,
    )
    from concourse.dve_spec import C0, C1, C2, Spec, Src0, lower, minn, relu, sq
    from concourse.dve_uop import DveOpSpec

    def reg(name, spec):
        for op in OPS:
            if op.name == name:
                return op
        row = _CUSTOM_DVE_ROW_BASE + len(OPS)
        assert row < 0x20
        _SUB_OPCODE_FOR_NAME[name] = row
        shas = {}
        for ver in ("v3", "v4"):
            s = DveOpSpec(name=name, opcode=row, uops=lower(spec, ver=ver),
                          rd1_en=False)
            shas[ver] = s.sha(ver)
        op = DveOp(name, spec, subdim=False, uops_sha=shas)
        OPS.append(op)
        CUSTOM_DVE_SPECS[name] = spec
        return op

    # r = relu(t)^2 * t  with t = min(x, C1)*C0 + C2  (= relu(t)^3). With
    # C1=+big this is a plain affine relu-cube of the pre-clamped x; C0
    # carries +-inv_h (direct vs mirrored cubes), C2 the per-shift offset.
    _t = minn(Src0, C1) * C0 + C2
    clamp_cube = Spec(
        body=sq(relu(_t)) * _t,
        reference=lambda in0, in1, s0, s1, imm2: (
            lambda t: (np.maximum(t, 0.0) ** 2 * t)
        )(np.minimum(in0, s1) * s0 + imm2).astype(np.float32),
    )
    return reg("ANT_KAN_CLAMP_CUBE", clamp_cube)


OP_CLAMP_CUBE = _register_ops()

BIG = 3.0e38  # C1 value that disables the fused min-clamp

# ---------------------------------------------------------------- device kernel

_NC_CACHE = {}


def _build_nc():
    if "nc" in _NC_CACHE:
        return _NC_CACHE["nc"]
    nc = bacc.Bacc("TRN2", target_bir_lowering=False)
    xT = nc.dram_tensor("xT", [IN_DIM, B_CORE], F32, kind="ExternalInput")
    # weights pre-arranged on host as [i, chunk, k]:
    #   wfE chunks (exact fp32): u, u^2, u^3, rho_5, r_6
    #   wfR chunks (tf32-rounded): silu, rho_1..4, r_7..10
    wfE = nc.dram_tensor("wfE", [IN_DIM, NEF, OUT_DIM], F32, kind="ExternalInput")
    wfR = nc.dram_tensor("wfR", [IN_DIM, NRF, OUT_DIM], F32R, kind="ExternalInput")
    bv = nc.dram_tensor("bv", [OUT_DIM, 1], F32, kind="ExternalInput")
    yT = nc.dram_tensor("yT", [OUT_DIM, B_CORE], F32, kind="ExternalOutput")

    inv_h = _NC_CACHE["inv_h"]
    xmin = _NC_CACHE["xmin"]
    xmax = _NC_CACHE["xmax"]
    u_off = _NC_CACHE["u_off"]

    with TileContext(nc) as tc:
        with (
            tc.tile_pool(name="wpool", bufs=1) as wpool,
            tc.tile_pool(name="dpool", bufs=1) as dpool,
            tc.tile_pool(name="ppool", bufs=2, space="PSUM") as ppool,
            tc.tile_pool(name="wppool", bufs=1, space="PSUM") as wppool,
        ):
            # input DMAs in first-use order; packets round-robin across the
            # descriptors, so earlier descriptors still finish first
            xt = dpool.tile([IN_DIM, B_CORE], F32, tag="xt")
            wtR = wpool.tile([IN_DIM, NRF, OUT_DIM], F32R, tag="wtR")
            wtE = wpool.tile([IN_DIM, NEF, OUT_DIM], F32, tag="wtE")
            bvt = wpool.tile([OUT_DIM, 1], F32, tag="bvt")
            nc.sync.dma_start(out=xt[:, 0:HB], in_=xT[:, 0:HB])
            nc.sync.dma_start(out=xt[:, HB:B_CORE], in_=xT[:, HB:B_CORE])
            nc.sync.dma_start(out=wtR[:, 0:3, :], in_=wfR[:, 0:3, :])
            nc.sync.dma_start(out=wtE[:, 0:2, :], in_=wfE[:, 0:2, :])
            nc.sync.dma_start(out=wtR[:, 3:NRF, :], in_=wfR[:, 3:NRF, :])
            nc.sync.dma_start(out=wtE[:, 2:NEF, :], in_=wfE[:, 2:NEF, :])
            nc.sync.dma_start(out=bvt[:], in_=bv[:])

            # PE warm-up: the HAM clock gate keeps the PE at 1.2 GHz until
            # ~3.4us of sustained activity. Burn dummy matmuls on a zeroed
            # scratch tile during the input-DMA wait.
            warm = dpool.tile([IN_DIM, 640], F32, tag="warm")
            nc.gpsimd.memset(warm[:], 0.0)
            # per-partition bias constants for the Scalar-engine affines
            bc = dpool.tile([IN_DIM, 3], F32, tag="bc")
            nc.gpsimd.memset(bc[:, 0:1], u_off)
            nc.gpsimd.memset(bc[:, 1:2], 5.0 - u_off)
            nc.gpsimd.memset(bc[:, 2:3], u_off - 6.0)
            psw = wppool.tile([OUT_DIM, HB], F32, tag="psw")
            for _ in range(3):
                nc.tensor.matmul(psw[:], lhsT=warm[:, 0:128],
                                 rhs=warm[:, 128:640], start=True, stop=True)

            featE = dpool.tile([IN_DIM, NEF, B_CORE], F32, tag="featE")
            featR = dpool.tile([IN_DIM, NRF, B_CORE], F32R, tag="featR")
            xc = dpool.tile([IN_DIM, B_CORE], F32, tag="xc")
            q5 = dpool.tile([IN_DIM, B_CORE], F32, tag="q5")
            p5 = dpool.tile([IN_DIM, B_CORE], F32, tag="p5")
            q6 = dpool.tile([IN_DIM, B_CORE], F32, tag="q6")
            p6 = dpool.tile([IN_DIM, B_CORE], F32, tag="p6")
            yt = dpool.tile([OUT_DIM, B_CORE], F32, tag="yt")
            ps0 = ppool.tile([OUT_DIM, HB], F32, tag="ps0")
            ps1 = ppool.tile([OUT_DIM, HB], F32, tag="ps1")
            ps = [ps0, ps1]
            MUL = mybir.AluOpType.mult

            for h in range(2):
                lo, hi = h * HB, (h + 1) * HB

                # DVE first in program order: clamp (the Scalar affines and
                # all cubes read xc)
                nc.vector.tensor_scalar(xc[:, lo:hi], xt[:, lo:hi],
                                        xmax, xmin,
                                        mybir.AluOpType.min,
                                        mybir.AluOpType.max)
                # Scalar engine queue for this half
                nc.scalar.activation(featR[:, 0, lo:hi], xt[:, lo:hi],
                                     AF.Silu)
                nc.scalar.activation(featE[:, 0, lo:hi], xc[:, lo:hi],
                                     AF.Identity, bias=bc[:, 0:1],
                                     scale=inv_h)
                nc.scalar.activation(featE[:, 1, lo:hi], xc[:, lo:hi],
                                     AF.Square, bias=bc[:, 0:1],
                                     scale=inv_h)
                nc.scalar.activation(q5[:, lo:hi], xc[:, lo:hi], AF.Square,
                                     bias=bc[:, 1:2], scale=-inv_h)
                nc.scalar.activation(p5[:, lo:hi], xc[:, lo:hi], AF.Relu,
                                     bias=bc[:, 1:2], scale=-inv_h)
                nc.scalar.activation(q6[:, lo:hi], xc[:, lo:hi], AF.Square,
                                     bias=bc[:, 2:3], scale=inv_h)
                nc.scalar.activation(p6[:, lo:hi], xc[:, lo:hi], AF.Relu,
                                     bias=bc[:, 2:3], scale=inv_h)
                # DVE: 8 fused relu-cubes
                for s in (1, 2, 3, 4):      # mirrored rho_s -> featR[1..4]
                    nc.vector._custom_dve(
                        OP_CLAMP_CUBE, out=featR[:, s, lo:hi],
                        in0=xc[:, lo:hi],
                        s0=-inv_h, s1=BIG, imm2=float(s) - u_off,
                    )
                for s in (7, 8, 9, 10):     # direct r_s -> featR[5..8]
                    nc.vector._custom_dve(
                        OP_CLAMP_CUBE, out=featR[:, s - 2, lo:hi],
                        in0=xc[:, lo:hi],
                        s0=inv_h, s1=BIG, imm2=u_off - float(s),
                    )
                # GpSimd: the three products
                nc.gpsimd.tensor_tensor(featE[:, 2, lo:hi],
                                        featE[:, 1, lo:hi],
                                        featE[:, 0, lo:hi], MUL)   # u^3
                nc.gpsimd.tensor_tensor(featE[:, 3, lo:hi],
                                        q5[:, lo:hi], p5[:, lo:hi], MUL)
                nc.gpsimd.tensor_tensor(featE[:, 4, lo:hi],
                                        q6[:, lo:hi], p6[:, lo:hi], MUL)

            for h in range(2):
                lo, hi = h * HB, (h + 1) * HB
                # matmul chain for this half's PSUM bank, availability
                # order with same-dtype runs grouped; cheap tf32 tail
                chain = [
                    (wtR[:, 0, :], featR[:, 0, lo:hi]),   # silu   R
                    (wtR[:, 1, :], featR[:, 1, lo:hi]),   # rho_1  R
                    (wtE[:, 0, :], featE[:, 0, lo:hi]),   # u      E
                    (wtE[:, 1, :], featE[:, 1, lo:hi]),   # u^2    E
                    (wtR[:, 2, :], featR[:, 2, lo:hi]),   # rho_2  R
                    (wtR[:, 3, :], featR[:, 3, lo:hi]),   # rho_3  R
                    (wtR[:, 4, :], featR[:, 4, lo:hi]),   # rho_4  R
                    (wtR[:, 5, :], featR[:, 5, lo:hi]),   # r_7    R
                    (wtE[:, 2, :], featE[:, 2, lo:hi]),   # u^3    E
                    (wtE[:, 3, :], featE[:, 3, lo:hi]),   # rho_5  E
                    (wtE[:, 4, :], featE[:, 4, lo:hi]),   # r_6    E
                    (wtR[:, 6, :], featR[:, 6, lo:hi]),   # r_8    R
                    (wtR[:, 7, :], featR[:, 7, lo:hi]),   # r_9    R
                    (wtR[:, 8, :], featR[:, 8, lo:hi]),   # r_10   R (tail)
                ]
                for ci, (w, f) in enumerate(chain):
                    nc.tensor.matmul(
                        ps[h][:], lhsT=w, rhs=f,
                        start=(ci == 0), stop=(ci == len(chain) - 1),
                    )
            # evacuate (+ folded bias) and ship; h0 on Scalar (after all of
            # its feature ops), h1 on Vector
            nc.scalar.activation(yt[:, 0:HB], ps0[:], AF.Identity,
                                 bias=bvt[:])
            nc.sync.dma_start(out=yT[:, 0:HB], in_=yt[:, 0:HB])
            nc.vector.tensor_scalar(yt[:, HB:B_CORE], ps1[:], bvt[:], None,
                                    mybir.AluOpType.add)
            nc.sync.dma_start(out=yT[:, HB:B_CORE], in_=yt[:, HB:B_CORE])

    nc.finalize()
    _NC_CACHE["nc"] = nc
    return nc


# ---------------------------------------------------------------- host wrapper


def _tf32_round(a):
    """Round fp32 to the tf32 grid (10 explicit mantissa bits, RNE)."""
    u = np.ascontiguousarray(a, np.float32).view(np.uint32)
    r = ((u.astype(np.uint64) + 0x1000 + ((u >> 13) & 1)) & 0xFFFFE000)
    return r.astype(np.uint32).view(np.float32)


def _build_weights(grid, spline_coeff, base_weight, scale):
    g0 = float(grid[0, 0])
    h = float(grid[0, 1] - grid[0, 0])
    sc = float(scale.reshape(-1)[0])
    # D[j, s]: coefficient of relu(u-s)^3 in B3(u-j), s <= NS-1
    D = np.zeros((NB, NS), dtype=np.float64)
    for j in range(NB):
        for m in range(5):
            s = j + m
            if s < NS:
                D[j, s] = (-1.0) ** m * math.comb(4, m) / 6.0
    C2 = np.einsum("ikj,js->iks", spline_coeff.astype(np.float64), D) * sc
    sv = np.arange(float(NMIR))
    w_u1 = (C2[:, :, :NMIR] * (3.0 * sv ** 2)).sum(-1)
    w_u2 = (C2[:, :, :NMIR] * (-3.0 * sv)).sum(-1)
    w_u3 = C2[:, :, :NMIR].sum(-1)
    bias_k = (C2[:, :, :NMIR] * (-sv ** 3)).sum(-1).sum(0)  # [k]

    wE = np.empty((IN_DIM, NEF, OUT_DIM), dtype=np.float32)
    wE[:, 0, :] = w_u1
    wE[:, 1, :] = w_u2
    wE[:, 2, :] = w_u3
    wE[:, 3, :] = C2[:, :, 5]
    wE[:, 4, :] = C2[:, :, 6]
    wR = np.empty((IN_DIM, NRF, OUT_DIM), dtype=np.float32)
    wR[:, 0, :] = base_weight.astype(np.float64) * sc
    for i, s in enumerate((1, 2, 3, 4)):
        wR[:, 1 + i, :] = C2[:, :, s]
    for i, s in enumerate((7, 8, 9, 10)):
        wR[:, 5 + i, :] = C2[:, :, s]
    return wE, _tf32_round(wR), bias_k.astype(np.float32), g0, h


def _prepare(x, grid, spline_coeff, base_weight, scale):
    """Build (nc, in_maps) for run_bass_kernel_spmd from full inputs."""
    wE, wR, bias_k, g0, h = _build_weights(grid, spline_coeff, base_weight,
                                           scale)
    _NC_CACHE.setdefault("inv_h", 1.0 / h)           # 2.5
    _NC_CACHE.setdefault("xmin", g0)                 # -2.2 (clamp: u >= 0)
    _NC_CACHE.setdefault("xmax", g0 + 11.0 * h)      # 2.2  (clamp: u <= 11)
    _NC_CACHE.setdefault("u_off", -g0 / h)           # 5.5

    nc = _build_nc()

    xT = np.ascontiguousarray(np.asarray(x).astype(np.float32).T)  # [128, 8192]
    bvv = np.ascontiguousarray(bias_k.reshape(OUT_DIM, 1))
    in_maps = []
    for c in range(N_CORES):
        in_maps.append({
            "xT": np.ascontiguousarray(xT[:, c * B_CORE:(c + 1) * B_CORE]),
            "wfE": wE,
            "wfR": wR,
            "bv": bvv,
        })
    return nc, in_maps


def kernel(x, grid, spline_coeff, base_weight, scale):
    from concourse.bass_utils import run_bass_kernel_spmd

    nc, in_maps = _prepare(x, grid, spline_coeff, base_weight, scale)
    res = run_bass_kernel_spmd(nc, in_maps, core_ids=list(range(N_CORES)))
    outs = res.results
    yT = np.concatenate([outs[c]["yT"] for c in range(N_CORES)], axis=1)
    return np.ascontiguousarray(yT.T)


if __name__ == "__main__":
    rng = np.random.default_rng(0)
    x = rng.standard_normal((B_TOTAL, IN_DIM)).astype(np.float32)
    g = np.linspace(-1, 1, 6)
    hh = 0.4
    for _ in range(3):
        g = np.concatenate([[g[0] - hh], g, [g[-1] + hh]])
    grid = np.broadcast_to(g.astype(np.float32), (IN_DIM, 12)).copy()
    C = rng.standard_normal((IN_DIM, OUT_DIM, NB)).astype(np.float32)
    W = rng.standard_normal((IN_DIM, OUT_DIM)).astype(np.float32)
    s = np.ones((1,), np.float32)
    y = kernel(x, grid, C, W, s)
    print(y.shape, y.dtype, np.abs(y).max())
